# revision 33
# baseline (speedup 1.0000x reference)
# Trainium2 Bass kernel for nn_DecoderBlock (masked self-attn + cross-attn +
# LFFN decoder block with "linear" softmax attention over the head dim).
#
# Sharding: data-parallel over batch — 16 batch elems / 8 cores = 2 per core.
# All weights replicated per core (bf16); activations stream per batch elem.
#
# Math per core/batch elem (validated against the jax reference in numpy):
#   per head: Q/K/V = x @ W[h]        ([s, dq] layout, s on partitions)
#   expQ/expK = exp((Q|K)/DQ**0.25)   (mask added to Q rows < 127 first)
#   V' = V * (1/rowsum(expK))         (folds K-softmax denominator)
#   A  = expK^T @ V'                  ([dq, dq])
#   softQ = expQ * (1/rowsum(expQ));  softQT = PE-transpose(softQ)   [dq, s]
#   BmT = A^T @ softQT                ([dq, s])
#   out rows [128h:128h+128] = sum_j BmT[:, j::8].T @ Wo.T[128j:128j+128, :]
#     (replicates the module's raw [b,h,s,d] -> [b, s, h*d] view)
#   residual + layernorm in natural [s, D] layout; transposed copy of the LN
#   output is produced on the PE for the next phase's lhsT operands.
#
# All weights are host-packed into [128, ...] images so each group loads with
# ONE big DMA; all transposes run on the TensorE (identity matmul) instead of
# the descriptor-bound DMA-transpose path.
import numpy as np
import ml_dtypes

import concourse.bacc as bacc
import concourse.mybir as mybir
import concourse.tile as tile
from concourse.bass_utils import run_bass_kernel_spmd
from concourse.masks import make_identity

H, D, DQ, BNK, HID = 8, 1024, 128, 512, 1024
B, S_T, S_M = 16, 1024, 2048
SCALE = DQ ** 0.25
EPS = 1e-5
NEG = -200.0 * 16  # pre-scaled: Q psum carries 16*Q
N_CORES = 8
BPC = B // N_CORES  # batch elems per core

f32 = mybir.dt.float32
bf16 = mybir.dt.bfloat16
fp8 = mybir.dt.float8e4
PM_DR = mybir.MatmulPerfMode.DoubleRow
AF = mybir.ActivationFunctionType
ALU = mybir.AluOpType
bf = ml_dtypes.bfloat16
e4 = ml_dtypes.float8_e4m3fn
WSC = 16.0  # host weight scale into fp8 range


def _build(affine: bool):
    nc = bacc.Bacc("TRN2", target_bir_lowering=False, debug=False,
                   enable_asserts=True, num_devices=N_CORES)

    def din(name, shape, dt=fp8):
        return nc.dram_tensor(name, list(shape), dt, kind="ExternalInput").ap()

    y0b = din("y0b", [BPC, S_T, D], bf16)            # natural bf16 (residual)
    y0T = din("y0T", [BPC, 128, 8, S_T])             # [b][128][kchunk][S_T]
    memTp = din("memTp", [BPC, 8, 128, 2, 8, 128])   # [b][jpair][p][i][k][q]
    wqkv1 = din("wqkv1", [128, 3, 2, 8, 512])        # [p][qkv][hg][kchunk][512]
    wqkv2 = din("wqkv2", [128, 3, 2, 8, 512])
    wo1t = din("wo1t", [128, 8, D])                  # [p][j][D]
    wo2t = din("wo2t", [128, 8, D])
    e1w = din("e1w", [128, 8, 4, 128])               # [p][kchunk][bn_tile][q]
    d1w = din("d1w", [128, 4, 8, 128])               # [p][bn_chunk][hid_tile][q]
    e2w = din("e2w", [128, 8, 4, 128])               # [p][hid_chunk][bn_tile][q]
    d2w = din("d2w", [128, 4, D])                    # [p][bn_chunk][D]
    mask4 = din("mask4", [128, 512], f32)
    grep = din("grep", [6, 128, D], f32) if affine else None

    out = nc.dram_tensor("out", [BPC, S_T, D], f32, kind="ExternalOutput").ap()

    with tile.TileContext(nc) as tc:
        with tc.tile_pool(name="dram", bufs=1, space="DRAM") as dpool:
            y1d = dpool.tile([BPC, S_T, D], bf16)
            y2d = dpool.tile([BPC, S_T, D], bf16)

            with tc.tile_pool(name="consts", bufs=1) as cpool:
                maskt = cpool.tile([128, 512], f32, tag="maskt")
                nc.sync.dma_start(maskt[:], mask4[:])
                eps_t = cpool.tile([128, 1], f32, tag="eps_t")
                nc.vector.memset(eps_t[:], EPS)
                ident = cpool.tile([128, 128], bf16, tag="ident")
                make_identity(nc, ident[:])
                gb = None
                if affine:
                    gb = [cpool.tile([128, D], f32, tag=f"gb{i}", name=f"gb{i}")
                          for i in range(6)]
                    for i in range(6):
                        nc.sync.dma_start(gb[i][:], grep[i])

                # persistent transposed-activation pool: one [128, 8, S_T]
                # tile per generation, 3 rotating buffers (y1T b0, y1T b1,
                # y2T b0; y2T b1 reuses y1T b0's buffer after last read)
                with tc.tile_pool(name="xT", bufs=1) as xpool, \
                     tc.tile_pool(name="wstg", bufs=1) as wpool:
                    def xt_alloc():
                        return xpool.tile([128, 8, S_T], fp8, tag="xT",
                                          name="xT", bufs=3)

                    def stg_kv(wqkv):
                        # staged K/V slab for the NEXT attn phase; the load
                        # overlaps the previous phase (tile WAR, not pool
                        # barrier, orders it)
                        stg = wpool.tile([128, 16384], fp8, tag="wstage",
                                         name="wstage", bufs=1)
                        kv = stg[:].rearrange("p (x h k q) -> p x h k q",
                                              x=2, h=2, k=8)
                        nc.sync.dma_start(kv[:, :, 0], wqkv[:, 1:3, 0])
                        nc.sync.dma_start(kv[:, :, 1], wqkv[:, 1:3, 1])
                        return kv

                    y1T = [None] * BPC
                    y2T = [None] * BPC
                    ctx = dict(nc=nc, tc=tc, maskt=maskt, eps_t=eps_t,
                               ident=ident, gb=gb)

                    kv1 = stg_kv(wqkv1)
                    _phase_attn(ctx, masked=True, xq_dram=y0T, memT=None,
                                wqkv=wqkv1, wot=wo1t, kv=kv1, res_d=y0b,
                                y_next_d=y1d, xT_in=None, xT_out=y1T,
                                xt_alloc=xt_alloc, gbi=0)
                    kv2 = stg_kv(wqkv2)
                    _phase_attn(ctx, masked=False, xq_dram=None, memT=memTp,
                                wqkv=wqkv2, wot=wo2t, kv=kv2, res_d=y1d,
                                y_next_d=y2d, xT_in=y1T, xT_out=y2T,
                                xt_alloc=xt_alloc, gbi=2)
                    stg = wpool.tile([128, 16384], fp8, tag="wstage",
                                     name="wstage", bufs=1)
                    e1v = stg[:, 0:4096].rearrange("p (k t q) -> p k t q",
                                                   k=8, t=4)
                    d1v = stg[:, 4096:8192].rearrange("p (c t q) -> p c t q",
                                                      c=4, t=8)
                    nc.sync.dma_start(e1v, e1w[:])
                    nc.sync.dma_start(d1v, d1w[:])
                    _phase_lffn(ctx, y2T, e1v, d1v, e2w, d2w, y2d, out, gbi=4)

    nc.compile()
    return nc


def _layernorm(ctx, pool, rsd, dst_dram, gbi, out_dt):
    """LN over the free axis of rsd [128, D] f32; write `out_dt` tile to
    dst_dram and return the SBUF tile."""
    nc, eps_t, gb = ctx["nc"], ctx["eps_t"], ctx["gb"]
    st6 = pool.tile([128, 2, 6], f32, tag="ln_st6", bufs=2)
    mv = pool.tile([128, 2], f32, tag="ln_mv", bufs=2)
    nc.vector.bn_stats(st6[:, 0, :], rsd[:, 0:512])
    nc.vector.bn_stats(st6[:, 1, :], rsd[:, 512:1024])
    nc.vector.bn_aggr(mv[:], st6[:])
    sd = pool.tile([128, 1], f32, tag="ln_sd", bufs=2)
    nc.scalar.activation(sd[:], mv[:, 1:2], AF.Sqrt, bias=eps_t[:])
    rstd = pool.tile([128, 1], f32, tag="ln_rstd", bufs=2)
    nc.vector.reciprocal(rstd[:], sd[:])
    cneg = pool.tile([128, 1], f32, tag="ln_cneg", bufs=2)
    nc.vector.scalar_tensor_tensor(
        out=cneg[:], in0=mv[:, 0:1], scalar=-1.0, in1=rstd[:],
        op0=ALU.mult, op1=ALU.mult)
    yt = pool.tile([128, D], out_dt, tag="ln_out", bufs=4)
    nc.scalar.activation(yt[:], rsd[:], AF.Identity, scale=rstd[:], bias=cneg[:])
    if gb is not None:
        g_t, b_t = gb[gbi], gb[gbi + 1]
        nc.vector.tensor_tensor(out=yt[:], in0=yt[:], in1=g_t[:], op=ALU.mult)
        nc.vector.tensor_tensor(out=yt[:], in0=yt[:], in1=b_t[:], op=ALU.add)
    nc.scalar.dma_start(dst_dram, yt[:])
    return yt


def _phase_attn(ctx, masked, xq_dram, memT, wqkv, wot, kv, res_d, y_next_d,
                xT_in, xT_out, xt_alloc, gbi):
    """One attention phase (self or cross) for all batch elems.

    Weights load on the sync HWDGE ring (K/V slab first so stage A can start
    early); activations/residuals use the scalar ring so the two FIFOs don't
    serialize each other.
    """
    nc, tc, ident = ctx["nc"], ctx["tc"], ctx["ident"]
    n_kv = 8 if memT is None else 16
    with tc.tile_pool(name="attn_sb", bufs=1) as sb:
        wq = sb.tile([128, 2, 8, 512], fp8, tag="wq")
        nc.sync.dma_start(wq[:], wqkv[:, 0])
        wo = sb.tile([128, 8, D], fp8, tag="wo")
        nc.sync.dma_start(wo[:], wot[:])

        xqs = [None] * BPC
        if xq_dram is not None:
            for b in range(BPC):
                xqs[b] = sb.tile([128, 8, S_T], fp8, tag="xq", bufs=2,
                                 name="xq")
                nc.scalar.dma_start(xqs[b][:], xq_dram[b])
        else:
            xqs = xT_in

        with tc.tile_pool(name="attn_ps", bufs=1, space="PSUM") as ps:
            for b in range(BPC):
                xq = xqs[b]

                xt_next = xt_alloc()
                xT_out[b] = xt_next
                for hg in range(2):
                    # ---- stage A: K/V proj + exp/evac + A accumulation ----
                    expk = sb.tile([128, n_kv, 512], bf16, tag="expk")
                    expv = sb.tile([128, n_kv, 512], bf16, tag="expv")
                    for j in range(n_kv // 2):
                        if memT is not None:
                            mt = sb.tile([128, 2, 8, 128], fp8, tag="mt",
                                         bufs=4)
                            nc.gpsimd.dma_start(mt[:], memT[b, j])
                        for i in range(2):
                            sm = 2 * j + i
                            kps = ps.tile([128, 512], f32, tag="ps512", bufs=4)
                            vps = ps.tile([128, 512], f32, tag="ps512", bufs=4)
                            for k in range(0, 8, 2):
                                if memT is None:
                                    lhsT = xq[:, k:k + 2, 128 * sm:128 * (sm + 1)]
                                else:
                                    lhsT = mt[:, i, k:k + 2, :]
                                nc.tensor.matmul(kps[:], lhsT,
                                                 kv[:, 0, hg, k:k + 2, :],
                                                 start=(k == 0), stop=(k == 6),
                                                 perf_mode=PM_DR)
                                nc.tensor.matmul(vps[:], lhsT,
                                                 kv[:, 1, hg, k:k + 2, :],
                                                 start=(k == 0), stop=(k == 6),
                                                 perf_mode=PM_DR)
                            nc.scalar.activation(expk[:, sm, :], kps[:], AF.Exp,
                                                 scale=1.0 / (WSC * SCALE))
                            krs = sb.tile([128, 4], f32, tag="krs", bufs=2)
                            nc.vector.tensor_reduce(
                                out=krs[:],
                                in_=expk[:, sm, :].rearrange("p (h q) -> p h q", h=4),
                                axis=mybir.AxisListType.X, op=ALU.add)
                            krr = sb.tile([128, 4], f32, tag="krr", bufs=2)
                            nc.vector.reciprocal(krr[:], krs[:])
                            nc.vector.tensor_tensor(
                                out=expv[:, sm, :].rearrange("p (h q) -> p h q", h=4),
                                in0=vps[:].rearrange("p (h q) -> p h q", h=4),
                                in1=krr[:].unsqueeze(2).broadcast_to([128, 4, 128]),
                                op=ALU.mult)
                    # ---- stage B Q proj (fills PE while stage-A evacs
                    # drain), then the A accumulation, then transposes ----
                    softqT = sb.tile([128, 4, S_T], bf16, tag="softqT", bufs=1)
                    sqa = sb.tile([128, 8, 4, 128], bf16, tag="sqa", bufs=1)
                    for st in range(8):
                        qps = ps.tile([128, 512], f32, tag="ps512", bufs=4)
                        for k in range(0, 8, 2):
                            nc.tensor.matmul(
                                qps[:], xq[:, k:k + 2, 128 * st:128 * (st + 1)],
                                wq[:, hg, k:k + 2, :], start=(k == 0),
                                stop=(k == 6), perf_mode=PM_DR)
                        if masked and st == 0:
                            nc.vector.tensor_tensor(
                                out=qps[:], in0=qps[:], in1=ctx["maskt"][:],
                                op=ALU.add)
                        eq = sb.tile([128, 512], f32, tag="eq", bufs=3)
                        nc.scalar.activation(eq[:], qps[:], AF.Exp,
                                             scale=1.0 / (WSC * SCALE))
                        qrs = sb.tile([128, 4], f32, tag="qrs", bufs=2)
                        nc.vector.tensor_reduce(
                            out=qrs[:], in_=eq[:].rearrange("p (h q) -> p h q", h=4),
                            axis=mybir.AxisListType.X, op=ALU.add)
                        qrr = sb.tile([128, 4], f32, tag="qrr", bufs=2)
                        nc.vector.reciprocal(qrr[:], qrs[:])
                        nc.vector.tensor_tensor(
                            out=sqa[:, st], in0=eq[:].rearrange("p (h q) -> p h q", h=4),
                            in1=qrr[:].unsqueeze(2).broadcast_to([128, 4, 128]),
                            op=ALU.mult)
                    # per-region accumulation groups must stay consecutive:
                    # interleaving groups within one PSUM bank corrupts them.
                    aps = ps.tile([128, 512], f32, tag="aps", bufs=1)
                    for hi in range(4):
                        for sm in range(n_kv):
                            nc.tensor.matmul(
                                aps[:, 128 * hi:128 * (hi + 1)],
                                expk[:, sm, 128 * hi:128 * (hi + 1)],
                                expv[:, sm, 128 * hi:128 * (hi + 1)],
                                start=(sm == 0), stop=(sm == n_kv - 1))
                    asb = sb.tile([128, 512], bf16, tag="asb", bufs=2)
                    nc.vector.tensor_copy(asb[:], aps[:])
                    for st in range(8):
                        tp = ps.tile([128, 8, 128], bf16, tag="tpb", bufs=1)
                        for hi in range(4):
                            nc.tensor.transpose(tp[:, hi, :], sqa[:, st, hi, :],
                                                ident[:])
                        nc.scalar.activation(
                            softqT[:, :, 128 * st:128 * (st + 1)],
                            tp[:, 0:4, :], AF.Identity)

                    # ---- stage C: Bm, Wo, residual + LN per head ----
                    nats, ybs = [], []
                    def _load_nat(hi):
                        hb = 4 * hg + hi
                        nat = sb.tile([128, D], bf16, tag="res_nat", bufs=3,
                                      name="res_nat")
                        nc.scalar.dma_start(
                            nat[:], res_d[b, 128 * hb:128 * (hb + 1), :])
                        nats.append(nat)
                    for hi in range(3):
                        _load_nat(hi)
                    for hi in range(4):
                        hb = 4 * hg + hi  # head == output s-tile block
                        # bms2[p, jj, m] = BmT(16x) at s = 8*m + jj, so the
                        # Wo contraction can pair jj-groups for DoubleRow
                        bms = sb.tile([128, 8, 128], fp8, tag="bms", bufs=2)
                        for half in range(2):
                            bmt = ps.tile([128, 512], f32, tag="ps512", bufs=4)
                            nc.tensor.matmul(bmt[:],
                                             asb[:, 128 * hi:128 * (hi + 1)],
                                             softqT[:, hi,
                                                    512 * half:512 * (half + 1)])
                            nc.vector.tensor_copy(
                                bms[:, :, 64 * half:64 * (half + 1)],
                                bmt[:].rearrange("p (m j) -> p j m", j=8))
                        ops = ps.tile([128, D], f32, tag="ps1k", bufs=1)
                        for jj in range(0, 8, 2):
                            for nh in range(2):
                                nc.tensor.matmul(
                                    ops[:, 512 * nh:512 * (nh + 1)],
                                    bms[:, jj:jj + 2, :],
                                    wo[:, jj:jj + 2, 512 * nh:512 * (nh + 1)],
                                    start=(jj == 0), stop=(jj == 6),
                                    perf_mode=PM_DR)
                        rsd = sb.tile([128, D], f32, tag="rsd", bufs=1)
                        nc.vector.scalar_tensor_tensor(
                            out=rsd[:], in0=ops[:], scalar=1.0 / (WSC * WSC),
                            in1=nats[hi][:], op0=ALU.mult, op1=ALU.add)
                        if hi == 0:
                            _load_nat(3)
                        yb = _layernorm(ctx, sb, rsd,
                                        y_next_d[b, 128 * hb:128 * (hb + 1), :],
                                        gbi, bf16)
                        ybs.append(yb)
                    # transposed copies for the next phase, batched so the
                    # PE never waits on an LN chain mid-stage
                    for hi in range(4):
                        hb = 4 * hg + hi
                        tp2 = ps.tile([128, 8, 128], bf16, tag="tpb", bufs=1)
                        for k in range(8):
                            nc.tensor.transpose(
                                tp2[:, k, :],
                                ybs[hi][:, 128 * k:128 * (k + 1)], ident[:])
                        nc.scalar.activation(
                            xt_next[:, :, 128 * hb:128 * (hb + 1)], tp2[:],
                            AF.Identity)


def _phase_lffn(ctx, y2T, e1w_d, d1w_d, e2w_d2, d2w_d2, y2d, out, gbi):
    nc, tc = ctx["nc"], ctx["tc"]
    with tc.tile_pool(name="ffn_sb", bufs=1) as sb:
        e1, d1 = e1w_d, d1w_d  # staged SBUF views
        e2 = sb.tile([128, 8, 4, 128], fp8, tag="e2")
        nc.sync.dma_start(e2[:], e2w_d2[:])
        d2 = sb.tile([128, 4, D], fp8, tag="d2")
        nc.sync.dma_start(d2[:], d2w_d2[:])

        with tc.tile_pool(name="ffn_ps", bufs=1, space="PSUM") as ps:
            for b in range(BPC):
                xT = y2T[b]  # [128, 8, S_T] fp8
                # h1T = E1 @ y2T  (16x scale)  [BN(4), S_T]
                h1T = sb.tile([128, 4, S_T], fp8, tag="h1T", bufs=2)
                for t_ in range(4):
                    acc = ps.tile([128, S_T], f32, tag="acc", bufs=2)
                    for nh in range(2):
                        for k in range(0, 8, 2):
                            nc.tensor.matmul(
                                acc[:, 512 * nh:512 * (nh + 1)],
                                e1[:, k:k + 2, t_, :],
                                xT[:, k:k + 2, 512 * nh:512 * (nh + 1)],
                                start=(k == 0), stop=(k == 6), perf_mode=PM_DR)
                    nc.vector.tensor_copy(h1T[:, t_], acc[:])
                # h2T = D1 @ h1T (256x) -> silu(x/256) -> swT (1x)
                swT = sb.tile([128, 8, S_T], fp8, tag="swT", bufs=2)
                for t_ in range(8):
                    acc = ps.tile([128, S_T], f32, tag="acc", bufs=2)
                    for nh in range(2):
                        for k in range(0, 4, 2):
                            nc.tensor.matmul(
                                acc[:, 512 * nh:512 * (nh + 1)],
                                d1[:, k:k + 2, t_, :],
                                h1T[:, k:k + 2, 512 * nh:512 * (nh + 1)],
                                start=(k == 0), stop=(k == 2), perf_mode=PM_DR)
                    nc.scalar.activation(swT[:, t_], acc[:], AF.Silu,
                                         scale=1.0 / (WSC ** 2))
                # g1T = E2 @ swT  (16x)
                g1T = sb.tile([128, 4, S_T], fp8, tag="g1T", bufs=2)
                for t_ in range(4):
                    acc = ps.tile([128, S_T], f32, tag="acc", bufs=2)
                    for nh in range(2):
                        for k in range(0, 8, 2):
                            nc.tensor.matmul(
                                acc[:, 512 * nh:512 * (nh + 1)],
                                e2[:, k:k + 2, t_, :],
                                swT[:, k:k + 2, 512 * nh:512 * (nh + 1)],
                                start=(k == 0), stop=(k == 6), perf_mode=PM_DR)
                    nc.vector.tensor_copy(g1T[:, t_], acc[:])
                # ffn[st] = g1T[:, st].T @ D2T (256x); residual; LN3 -> out
                for st in range(8):
                    nat = sb.tile([128, D], bf16, tag="y2res", bufs=2)
                    nc.scalar.dma_start(nat[:],
                                        y2d[b, 128 * st:128 * (st + 1), :])
                    acc = ps.tile([128, D], f32, tag="acc2", bufs=2)
                    for nh in range(2):
                        for k in range(0, 4, 2):
                            nc.tensor.matmul(
                                acc[:, 512 * nh:512 * (nh + 1)],
                                g1T[:, k:k + 2, 128 * st:128 * (st + 1)],
                                d2[:, k:k + 2, 512 * nh:512 * (nh + 1)],
                                start=(k == 0), stop=(k == 2), perf_mode=PM_DR)
                    rsd = sb.tile([128, D], f32, tag="rsd", bufs=2)
                    nc.vector.scalar_tensor_tensor(
                        out=rsd[:], in0=acc[:], scalar=1.0 / (WSC ** 2),
                        in1=nat[:], op0=ALU.mult, op1=ALU.add)
                    _layernorm(ctx, sb, rsd,
                               out[b, 128 * st:128 * (st + 1), :], gbi, f32)


_CACHE = {}


def _prep_host(inputs):
    """Convert/transpose/pack weights + activations per the kernel layout."""
    g = {k: np.asarray(v) for k, v in inputs.items()}
    affine = not (
        np.all(g["g1"] == 1) and np.all(g["g2"] == 1) and np.all(g["g3"] == 1)
        and np.all(g["b1"] == 0) and np.all(g["b2"] == 0) and np.all(g["b3"] == 0))

    def wqkv_pack(q, k, v):
        # [H, D, DQ] -> [p=128][qkv][hg][kchunk][512] (4 heads concat)
        def onev2(w):
            arr = np.empty((2, 8, 128, 512), np.float32)
            for hg in range(2):
                for kc in range(8):
                    cols = [w[4 * hg + hi, 128 * kc:128 * (kc + 1), :]
                            for hi in range(4)]
                    arr[hg, kc] = np.concatenate(cols, axis=1)
            return arr
        st = np.stack([onev2(q), onev2(k), onev2(v)])  # [3,2,8,128,512]
        return np.ascontiguousarray(st.transpose(3, 0, 1, 2, 4) * WSC).astype(e4)

    host = {}
    host["wqkv1"] = wqkv_pack(g["Wq1"], g["Wk1"], g["Wv1"])
    host["wqkv2"] = wqkv_pack(g["Wq2"], g["Wk2"], g["Wv2"])
    host["wo1t"] = np.ascontiguousarray(
        g["Wo1"].T.reshape(8, 128, D).transpose(1, 0, 2) * WSC).astype(e4)
    host["wo2t"] = np.ascontiguousarray(
        g["Wo2"].T.reshape(8, 128, D).transpose(1, 0, 2) * WSC).astype(e4)
    host["e1w"] = np.ascontiguousarray(
        g["E1"].T.reshape(8, 128, 4, 128).transpose(1, 0, 2, 3) * WSC).astype(e4)
    host["d1w"] = np.ascontiguousarray(
        g["D1"].T.reshape(4, 128, 8, 128).transpose(1, 0, 2, 3) * WSC).astype(e4)
    host["e2w"] = np.ascontiguousarray(
        g["E2"].T.reshape(8, 128, 4, 128).transpose(1, 0, 2, 3) * WSC).astype(e4)
    host["d2w"] = np.ascontiguousarray(
        g["D2"].T.reshape(4, 128, D).transpose(1, 0, 2) * WSC).astype(e4)
    mask = np.where(np.arange(DQ)[None, :] <= np.arange(128)[:, None],
                    0.0, NEG).astype(np.float32)
    host["mask4"] = np.tile(mask, (1, 4))
    if affine:
        host["grep"] = np.stack([
            np.broadcast_to(g[n].astype(np.float32), (128, D))
            for n in ("g1", "b1", "g2", "b2", "g3", "b3")]).copy()

    in_maps = []
    y = g["y"].astype(np.float32)
    mem = g["mem"].astype(np.float32)
    for c in range(N_CORES):
        sl = slice(BPC * c, BPC * (c + 1))
        m = dict(host)
        m["y0b"] = y[sl].astype(bf)
        yT = np.ascontiguousarray(y[sl].transpose(0, 2, 1)).astype(e4)
        m["y0T"] = np.ascontiguousarray(
            yT.reshape(BPC, 8, 128, S_T).transpose(0, 2, 1, 3))
        mT = mem[sl].transpose(0, 2, 1).astype(e4)  # [b, D, S_M]
        # [b, k, p, j, i, q] -> [b, j, p, i, k, q]
        m["memTp"] = np.ascontiguousarray(
            mT.reshape(BPC, 8, 128, 8, 2, 128).transpose(0, 3, 2, 4, 1, 5))
        in_maps.append(m)
    return in_maps, affine


def kernel(**inputs):
    in_maps, affine = _prep_host(inputs)
    if affine not in _CACHE:
        _CACHE[affine] = _build(affine)
    nc = _CACHE[affine]
    res = run_bass_kernel_spmd(nc, in_maps, list(range(N_CORES)))
    return np.concatenate([r["out"] for r in res.results], axis=0)


if __name__ == "__main__":
    rng = np.random.default_rng(0)
    ins = {
        "mem": rng.standard_normal((B, S_M, D), dtype=np.float32),
        "y": rng.standard_normal((B, S_T, D), dtype=np.float32),
        **{k: (rng.standard_normal(s, dtype=np.float32) * 0.02).astype(np.float32)
           for k, s in {
               "Wq1": (H, D, DQ), "Wk1": (H, D, DQ), "Wv1": (H, D, DQ),
               "Wo1": (D, D), "Wq2": (H, D, DQ), "Wk2": (H, D, DQ),
               "Wv2": (H, D, DQ), "Wo2": (D, D), "E1": (BNK, D),
               "D1": (HID, BNK), "E2": (BNK, HID), "D2": (D, BNK)}.items()},
        "g1": np.ones(D, np.float32), "b1": np.zeros(D, np.float32),
        "g2": np.ones(D, np.float32), "b2": np.zeros(D, np.float32),
        "g3": np.ones(D, np.float32), "b3": np.zeros(D, np.float32),
    }
    o = kernel(**ins)
    print("out", o.shape, o.dtype, np.abs(o).mean())


# revision 34
# speedup vs baseline: 1.0337x; 1.0337x over previous
# Trainium2 Bass kernel for nn_DecoderBlock (masked self-attn + cross-attn +
# LFFN decoder block with "linear" softmax attention over the head dim).
#
# Sharding: data-parallel over batch — 16 batch elems / 8 cores = 2 per core.
# All weights replicated per core (bf16); activations stream per batch elem.
#
# Math per core/batch elem (validated against the jax reference in numpy):
#   per head: Q/K/V = x @ W[h]        ([s, dq] layout, s on partitions)
#   expQ/expK = exp((Q|K)/DQ**0.25)   (mask added to Q rows < 127 first)
#   V' = V * (1/rowsum(expK))         (folds K-softmax denominator)
#   A  = expK^T @ V'                  ([dq, dq])
#   softQ = expQ * (1/rowsum(expQ));  softQT = PE-transpose(softQ)   [dq, s]
#   BmT = A^T @ softQT                ([dq, s])
#   out rows [128h:128h+128] = sum_j BmT[:, j::8].T @ Wo.T[128j:128j+128, :]
#     (replicates the module's raw [b,h,s,d] -> [b, s, h*d] view)
#   residual + layernorm in natural [s, D] layout; transposed copy of the LN
#   output is produced on the PE for the next phase's lhsT operands.
#
# All weights are host-packed into [128, ...] images so each group loads with
# ONE big DMA; all transposes run on the TensorE (identity matmul) instead of
# the descriptor-bound DMA-transpose path.
import numpy as np
import ml_dtypes

import concourse.bacc as bacc
import concourse.mybir as mybir
import concourse.tile as tile
from concourse.bass_utils import run_bass_kernel_spmd
from concourse.masks import make_identity

H, D, DQ, BNK, HID = 8, 1024, 128, 512, 1024
B, S_T, S_M = 16, 1024, 2048
SCALE = DQ ** 0.25
EPS = 1e-5
NEG = -200.0 * 16  # pre-scaled: Q psum carries 16*Q
N_CORES = 8
BPC = B // N_CORES  # batch elems per core

f32 = mybir.dt.float32
bf16 = mybir.dt.bfloat16
fp8 = mybir.dt.float8e4
PM_DR = mybir.MatmulPerfMode.DoubleRow
AF = mybir.ActivationFunctionType
ALU = mybir.AluOpType
bf = ml_dtypes.bfloat16
e4 = ml_dtypes.float8_e4m3fn
WSC = 16.0  # host weight scale into fp8 range


def _build(affine: bool):
    nc = bacc.Bacc("TRN2", target_bir_lowering=False, debug=False,
                   enable_asserts=True, num_devices=N_CORES)

    def din(name, shape, dt=fp8):
        return nc.dram_tensor(name, list(shape), dt, kind="ExternalInput").ap()

    y0b = din("y0b", [BPC, S_T, D], bf16)            # natural bf16 (residual)
    y0T = din("y0T", [BPC, 128, 8, S_T])             # [b][128][kchunk][S_T]
    memTp = din("memTp", [BPC, 8, 128, 2, 8, 128])   # [b][jpair][p][i][k][q]
    wqkv1 = din("wqkv1", [128, 3, 2, 8, 512])        # [p][qkv][hg][kchunk][512]
    wqkv2 = din("wqkv2", [128, 3, 2, 8, 512])
    wo1t = din("wo1t", [128, 8, D])                  # [p][j][D]
    wo2t = din("wo2t", [128, 8, D])
    e1w = din("e1w", [128, 8, 4, 128])               # [p][kchunk][bn_tile][q]
    d1w = din("d1w", [128, 4, 8, 128])               # [p][bn_chunk][hid_tile][q]
    e2w = din("e2w", [128, 8, 4, 128])               # [p][hid_chunk][bn_tile][q]
    d2w = din("d2w", [128, 4, D])                    # [p][bn_chunk][D]
    mask4 = din("mask4", [128, 512], f32)
    grep = din("grep", [6, 128, D], f32) if affine else None

    out = nc.dram_tensor("out", [BPC, S_T, D], f32, kind="ExternalOutput").ap()

    with tile.TileContext(nc) as tc:
        with tc.tile_pool(name="dram", bufs=1, space="DRAM") as dpool:
            y1d = dpool.tile([BPC, S_T, D], bf16)
            y2d = dpool.tile([BPC, S_T, D], bf16)

            with tc.tile_pool(name="consts", bufs=1) as cpool:
                maskt = cpool.tile([128, 512], f32, tag="maskt")
                nc.sync.dma_start(maskt[:], mask4[:])
                eps_t = cpool.tile([128, 1], f32, tag="eps_t")
                nc.vector.memset(eps_t[:], EPS)
                ident = cpool.tile([128, 128], bf16, tag="ident")
                make_identity(nc, ident[:])
                gb = None
                if affine:
                    gb = [cpool.tile([128, D], f32, tag=f"gb{i}", name=f"gb{i}")
                          for i in range(6)]
                    for i in range(6):
                        nc.sync.dma_start(gb[i][:], grep[i])

                # persistent transposed-activation pool: one [128, 8, S_T]
                # tile per generation, 3 rotating buffers (y1T b0, y1T b1,
                # y2T b0; y2T b1 reuses y1T b0's buffer after last read)
                with tc.tile_pool(name="xT", bufs=1) as xpool, \
                     tc.tile_pool(name="wstg", bufs=1) as wpool:
                    def xt_alloc():
                        return xpool.tile([128, 8, S_T], fp8, tag="xT",
                                          name="xT", bufs=3)

                    def stg_kv(wqkv):
                        # staged K/V slab for the NEXT attn phase; the load
                        # overlaps the previous phase (tile WAR, not pool
                        # barrier, orders it)
                        stg = wpool.tile([128, 16384], fp8, tag="wstage",
                                         name="wstage", bufs=1)
                        kv = stg[:].rearrange("p (x h k q) -> p x h k q",
                                              x=2, h=2, k=8)
                        nc.sync.dma_start(kv[:, :, 0], wqkv[:, 1:3, 0])
                        nc.sync.dma_start(kv[:, :, 1], wqkv[:, 1:3, 1])
                        return kv

                    y1T = [None] * BPC
                    y2T = [None] * BPC
                    ctx = dict(nc=nc, tc=tc, maskt=maskt, eps_t=eps_t,
                               ident=ident, gb=gb)

                    kv1 = stg_kv(wqkv1)
                    _phase_attn(ctx, masked=True, xq_dram=y0T, memT=None,
                                wqkv=wqkv1, wot=wo1t, kv=kv1, res_d=y0b,
                                y_next_d=y1d, xT_in=None, xT_out=y1T,
                                xt_alloc=xt_alloc, gbi=0)
                    kv2 = stg_kv(wqkv2)
                    _phase_attn(ctx, masked=False, xq_dram=None, memT=memTp,
                                wqkv=wqkv2, wot=wo2t, kv=kv2, res_d=y1d,
                                y_next_d=y2d, xT_in=y1T, xT_out=y2T,
                                xt_alloc=xt_alloc, gbi=2)
                    stg = wpool.tile([128, 16384], fp8, tag="wstage",
                                     name="wstage", bufs=1)
                    e1v = stg[:, 0:4096].rearrange("p (k t q) -> p k t q",
                                                   k=8, t=4)
                    d1v = stg[:, 4096:8192].rearrange("p (c t q) -> p c t q",
                                                      c=4, t=8)
                    nc.sync.dma_start(e1v, e1w[:])
                    nc.sync.dma_start(d1v, d1w[:])
                    _phase_lffn(ctx, y2T, e1v, d1v, e2w, d2w, y2d, out, gbi=4)

    nc.compile()
    return nc


def _layernorm(ctx, pool, rsd, dst_dram, gbi, out_dt):
    """LN over the free axis of rsd [128, D] f32; write `out_dt` tile to
    dst_dram and return the SBUF tile."""
    nc, eps_t, gb = ctx["nc"], ctx["eps_t"], ctx["gb"]
    st6 = pool.tile([128, 2, 6], f32, tag="ln_st6", bufs=2)
    mv = pool.tile([128, 2], f32, tag="ln_mv", bufs=2)
    nc.vector.bn_stats(st6[:, 0, :], rsd[:, 0:512])
    nc.vector.bn_stats(st6[:, 1, :], rsd[:, 512:1024])
    nc.vector.bn_aggr(mv[:], st6[:])
    sd = pool.tile([128, 1], f32, tag="ln_sd", bufs=2)
    nc.scalar.activation(sd[:], mv[:, 1:2], AF.Sqrt, bias=eps_t[:])
    rstd = pool.tile([128, 1], f32, tag="ln_rstd", bufs=2)
    nc.vector.reciprocal(rstd[:], sd[:])
    cneg = pool.tile([128, 1], f32, tag="ln_cneg", bufs=2)
    nc.vector.scalar_tensor_tensor(
        out=cneg[:], in0=mv[:, 0:1], scalar=-1.0, in1=rstd[:],
        op0=ALU.mult, op1=ALU.mult)
    yt = pool.tile([128, D], out_dt, tag="ln_out", bufs=4)
    nc.scalar.activation(yt[:], rsd[:], AF.Identity, scale=rstd[:], bias=cneg[:])
    if gb is not None:
        g_t, b_t = gb[gbi], gb[gbi + 1]
        nc.vector.tensor_tensor(out=yt[:], in0=yt[:], in1=g_t[:], op=ALU.mult)
        nc.vector.tensor_tensor(out=yt[:], in0=yt[:], in1=b_t[:], op=ALU.add)
    nc.scalar.dma_start(dst_dram, yt[:])
    return yt


def _phase_attn(ctx, masked, xq_dram, memT, wqkv, wot, kv, res_d, y_next_d,
                xT_in, xT_out, xt_alloc, gbi):
    """One attention phase (self or cross) for all batch elems.

    Weights load on the sync HWDGE ring (K/V slab first so stage A can start
    early); activations/residuals use the scalar ring so the two FIFOs don't
    serialize each other.
    """
    nc, tc, ident = ctx["nc"], ctx["tc"], ctx["ident"]
    n_kv = 8 if memT is None else 16
    with tc.tile_pool(name="attn_sb", bufs=1) as sb:
        wq = sb.tile([128, 2, 8, 512], fp8, tag="wq")
        nc.sync.dma_start(wq[:], wqkv[:, 0])
        wo = sb.tile([128, 8, D], fp8, tag="wo")
        nc.sync.dma_start(wo[:], wot[:])

        xqs = [None] * BPC
        if xq_dram is not None:
            for b in range(BPC):
                xqs[b] = sb.tile([128, 8, S_T], fp8, tag="xq", bufs=2,
                                 name="xq")
                nc.scalar.dma_start(xqs[b][:], xq_dram[b])
        else:
            xqs = xT_in

        with tc.tile_pool(name="attn_ps", bufs=1, space="PSUM") as ps:
            for b in range(BPC):
                xq = xqs[b]

                xt_next = xt_alloc()
                xT_out[b] = xt_next
                for hg in range(2):
                    # ---- stage A: K/V proj + exp/evac + A accumulation ----
                    expk = sb.tile([128, n_kv, 512], bf16, tag="expk")
                    expv = sb.tile([128, n_kv, 512], bf16, tag="expv")
                    for j in range(n_kv // 2):
                        if memT is not None:
                            mt = sb.tile([128, 2, 8, 128], fp8, tag="mt",
                                         bufs=4)
                            nc.gpsimd.dma_start(mt[:], memT[b, j])
                        for i in range(2):
                            sm = 2 * j + i
                            kps = ps.tile([128, 512], f32, tag="ps512", bufs=3)
                            vps = ps.tile([128, 512], f32, tag="ps512", bufs=3)
                            for k in range(0, 8, 2):
                                if memT is None:
                                    lhsT = xq[:, k:k + 2, 128 * sm:128 * (sm + 1)]
                                else:
                                    lhsT = mt[:, i, k:k + 2, :]
                                nc.tensor.matmul(kps[:], lhsT,
                                                 kv[:, 0, hg, k:k + 2, :],
                                                 start=(k == 0), stop=(k == 6),
                                                 perf_mode=PM_DR)
                                nc.tensor.matmul(vps[:], lhsT,
                                                 kv[:, 1, hg, k:k + 2, :],
                                                 start=(k == 0), stop=(k == 6),
                                                 perf_mode=PM_DR)
                            nc.scalar.activation(expk[:, sm, :], kps[:], AF.Exp,
                                                 scale=1.0 / (WSC * SCALE))
                            krs = sb.tile([128, 4], f32, tag="krs", bufs=2)
                            nc.vector.tensor_reduce(
                                out=krs[:],
                                in_=expk[:, sm, :].rearrange("p (h q) -> p h q", h=4),
                                axis=mybir.AxisListType.X, op=ALU.add)
                            krr = sb.tile([128, 4], f32, tag="krr", bufs=2)
                            nc.vector.reciprocal(krr[:], krs[:])
                            nc.vector.tensor_tensor(
                                out=expv[:, sm, :].rearrange("p (h q) -> p h q", h=4),
                                in0=vps[:].rearrange("p (h q) -> p h q", h=4),
                                in1=krr[:].unsqueeze(2).broadcast_to([128, 4, 128]),
                                op=ALU.mult)
                    # ---- stage B Q proj (fills PE while stage-A evacs
                    # drain), then the A accumulation, then transposes ----
                    softqT = sb.tile([128, 4, S_T], bf16, tag="softqT", bufs=1)
                    sqa = sb.tile([128, 8, 4, 128], bf16, tag="sqa", bufs=1)
                    for st in range(8):
                        qps = ps.tile([128, 512], f32, tag="ps512", bufs=3)
                        for k in range(0, 8, 2):
                            nc.tensor.matmul(
                                qps[:], xq[:, k:k + 2, 128 * st:128 * (st + 1)],
                                wq[:, hg, k:k + 2, :], start=(k == 0),
                                stop=(k == 6), perf_mode=PM_DR)
                        if masked and st == 0:
                            nc.vector.tensor_tensor(
                                out=qps[:], in0=qps[:], in1=ctx["maskt"][:],
                                op=ALU.add)
                        eq = sb.tile([128, 512], f32, tag="eq", bufs=3)
                        nc.scalar.activation(eq[:], qps[:], AF.Exp,
                                             scale=1.0 / (WSC * SCALE))
                        qrs = sb.tile([128, 4], f32, tag="qrs", bufs=2)
                        nc.vector.tensor_reduce(
                            out=qrs[:], in_=eq[:].rearrange("p (h q) -> p h q", h=4),
                            axis=mybir.AxisListType.X, op=ALU.add)
                        qrr = sb.tile([128, 4], f32, tag="qrr", bufs=2)
                        nc.vector.reciprocal(qrr[:], qrs[:])
                        nc.vector.tensor_tensor(
                            out=sqa[:, st], in0=eq[:].rearrange("p (h q) -> p h q", h=4),
                            in1=qrr[:].unsqueeze(2).broadcast_to([128, 4, 128]),
                            op=ALU.mult)
                    # per-region accumulation groups must stay consecutive:
                    # interleaving groups within one PSUM bank corrupts them.
                    aps = ps.tile([128, 512], f32, tag="aps", bufs=1)
                    for hi in range(4):
                        for sm in range(n_kv):
                            nc.tensor.matmul(
                                aps[:, 128 * hi:128 * (hi + 1)],
                                expk[:, sm, 128 * hi:128 * (hi + 1)],
                                expv[:, sm, 128 * hi:128 * (hi + 1)],
                                start=(sm == 0), stop=(sm == n_kv - 1))
                    asb = sb.tile([128, 512], bf16, tag="asb", bufs=2)
                    nc.vector.tensor_copy(asb[:], aps[:])
                    for st in range(8):
                        tp = ps.tile([128, 8, 128], bf16, tag="tpb", bufs=2)
                        for hi in range(4):
                            nc.tensor.transpose(tp[:, hi, :], sqa[:, st, hi, :],
                                                ident[:])
                        nc.scalar.activation(
                            softqT[:, :, 128 * st:128 * (st + 1)],
                            tp[:, 0:4, :], AF.Identity)

                    # ---- stage C: Bm, Wo, residual + LN per head ----
                    nats, ybs = [], []
                    def _load_nat(hi):
                        hb = 4 * hg + hi
                        nat = sb.tile([128, D], bf16, tag="res_nat", bufs=3,
                                      name="res_nat")
                        nc.scalar.dma_start(
                            nat[:], res_d[b, 128 * hb:128 * (hb + 1), :])
                        nats.append(nat)
                    for hi in range(3):
                        _load_nat(hi)
                    for hi in range(4):
                        hb = 4 * hg + hi  # head == output s-tile block
                        # bms2[p, jj, m] = BmT(16x) at s = 8*m + jj, so the
                        # Wo contraction can pair jj-groups for DoubleRow
                        bms = sb.tile([128, 8, 128], fp8, tag="bms", bufs=2)
                        for half in range(2):
                            bmt = ps.tile([128, 512], f32, tag="ps512", bufs=3)
                            nc.tensor.matmul(bmt[:],
                                             asb[:, 128 * hi:128 * (hi + 1)],
                                             softqT[:, hi,
                                                    512 * half:512 * (half + 1)])
                            nc.vector.tensor_copy(
                                bms[:, :, 64 * half:64 * (half + 1)],
                                bmt[:].rearrange("p (m j) -> p j m", j=8))
                        ops = ps.tile([128, D], f32, tag="ps1k", bufs=1)
                        for jj in range(0, 8, 2):
                            for nh in range(2):
                                nc.tensor.matmul(
                                    ops[:, 512 * nh:512 * (nh + 1)],
                                    bms[:, jj:jj + 2, :],
                                    wo[:, jj:jj + 2, 512 * nh:512 * (nh + 1)],
                                    start=(jj == 0), stop=(jj == 6),
                                    perf_mode=PM_DR)
                        rsd = sb.tile([128, D], f32, tag="rsd", bufs=1)
                        nc.vector.scalar_tensor_tensor(
                            out=rsd[:], in0=ops[:], scalar=1.0 / (WSC * WSC),
                            in1=nats[hi][:], op0=ALU.mult, op1=ALU.add)
                        if hi == 0:
                            _load_nat(3)
                        yb = _layernorm(ctx, sb, rsd,
                                        y_next_d[b, 128 * hb:128 * (hb + 1), :],
                                        gbi, bf16)
                        ybs.append(yb)
                    # transposed copies for the next phase, batched so the
                    # PE never waits on an LN chain mid-stage
                    for hi in range(4):
                        hb = 4 * hg + hi
                        tp2 = ps.tile([128, 8, 128], bf16, tag="tpb", bufs=2)
                        for k in range(8):
                            nc.tensor.transpose(
                                tp2[:, k, :],
                                ybs[hi][:, 128 * k:128 * (k + 1)], ident[:])
                        nc.scalar.activation(
                            xt_next[:, :, 128 * hb:128 * (hb + 1)], tp2[:],
                            AF.Identity)


def _phase_lffn(ctx, y2T, e1w_d, d1w_d, e2w_d2, d2w_d2, y2d, out, gbi):
    nc, tc = ctx["nc"], ctx["tc"]
    with tc.tile_pool(name="ffn_sb", bufs=1) as sb:
        e1, d1 = e1w_d, d1w_d  # staged SBUF views
        e2 = sb.tile([128, 8, 4, 128], fp8, tag="e2")
        nc.sync.dma_start(e2[:], e2w_d2[:])
        d2 = sb.tile([128, 4, D], fp8, tag="d2")
        nc.sync.dma_start(d2[:], d2w_d2[:])

        with tc.tile_pool(name="ffn_ps", bufs=1, space="PSUM") as ps:
            for b in range(BPC):
                xT = y2T[b]  # [128, 8, S_T] fp8
                # h1T = E1 @ y2T  (16x scale)  [BN(4), S_T]
                h1T = sb.tile([128, 4, S_T], fp8, tag="h1T", bufs=2)
                for t_ in range(4):
                    acc = ps.tile([128, S_T], f32, tag="acc", bufs=2)
                    for nh in range(2):
                        for k in range(0, 8, 2):
                            nc.tensor.matmul(
                                acc[:, 512 * nh:512 * (nh + 1)],
                                e1[:, k:k + 2, t_, :],
                                xT[:, k:k + 2, 512 * nh:512 * (nh + 1)],
                                start=(k == 0), stop=(k == 6), perf_mode=PM_DR)
                    nc.vector.tensor_copy(h1T[:, t_], acc[:])
                # h2T = D1 @ h1T (256x) -> silu(x/256) -> swT (1x)
                swT = sb.tile([128, 8, S_T], fp8, tag="swT", bufs=2)
                for t_ in range(8):
                    acc = ps.tile([128, S_T], f32, tag="acc", bufs=2)
                    for nh in range(2):
                        for k in range(0, 4, 2):
                            nc.tensor.matmul(
                                acc[:, 512 * nh:512 * (nh + 1)],
                                d1[:, k:k + 2, t_, :],
                                h1T[:, k:k + 2, 512 * nh:512 * (nh + 1)],
                                start=(k == 0), stop=(k == 2), perf_mode=PM_DR)
                    nc.scalar.activation(swT[:, t_], acc[:], AF.Silu,
                                         scale=1.0 / (WSC ** 2))
                # g1T = E2 @ swT  (16x)
                g1T = sb.tile([128, 4, S_T], fp8, tag="g1T", bufs=2)
                for t_ in range(4):
                    acc = ps.tile([128, S_T], f32, tag="acc", bufs=2)
                    for nh in range(2):
                        for k in range(0, 8, 2):
                            nc.tensor.matmul(
                                acc[:, 512 * nh:512 * (nh + 1)],
                                e2[:, k:k + 2, t_, :],
                                swT[:, k:k + 2, 512 * nh:512 * (nh + 1)],
                                start=(k == 0), stop=(k == 6), perf_mode=PM_DR)
                    nc.vector.tensor_copy(g1T[:, t_], acc[:])
                # ffn[st] = g1T[:, st].T @ D2T (256x); residual; LN3 -> out
                for st in range(8):
                    nat = sb.tile([128, D], bf16, tag="y2res", bufs=2)
                    nc.scalar.dma_start(nat[:],
                                        y2d[b, 128 * st:128 * (st + 1), :])
                    acc = ps.tile([128, D], f32, tag="acc2", bufs=2)
                    for nh in range(2):
                        for k in range(0, 4, 2):
                            nc.tensor.matmul(
                                acc[:, 512 * nh:512 * (nh + 1)],
                                g1T[:, k:k + 2, 128 * st:128 * (st + 1)],
                                d2[:, k:k + 2, 512 * nh:512 * (nh + 1)],
                                start=(k == 0), stop=(k == 2), perf_mode=PM_DR)
                    rsd = sb.tile([128, D], f32, tag="rsd", bufs=2)
                    nc.vector.scalar_tensor_tensor(
                        out=rsd[:], in0=acc[:], scalar=1.0 / (WSC ** 2),
                        in1=nat[:], op0=ALU.mult, op1=ALU.add)
                    _layernorm(ctx, sb, rsd,
                               out[b, 128 * st:128 * (st + 1), :], gbi, f32)


_CACHE = {}


def _prep_host(inputs):
    """Convert/transpose/pack weights + activations per the kernel layout."""
    g = {k: np.asarray(v) for k, v in inputs.items()}
    affine = not (
        np.all(g["g1"] == 1) and np.all(g["g2"] == 1) and np.all(g["g3"] == 1)
        and np.all(g["b1"] == 0) and np.all(g["b2"] == 0) and np.all(g["b3"] == 0))

    def wqkv_pack(q, k, v):
        # [H, D, DQ] -> [p=128][qkv][hg][kchunk][512] (4 heads concat)
        def onev2(w):
            arr = np.empty((2, 8, 128, 512), np.float32)
            for hg in range(2):
                for kc in range(8):
                    cols = [w[4 * hg + hi, 128 * kc:128 * (kc + 1), :]
                            for hi in range(4)]
                    arr[hg, kc] = np.concatenate(cols, axis=1)
            return arr
        st = np.stack([onev2(q), onev2(k), onev2(v)])  # [3,2,8,128,512]
        return np.ascontiguousarray(st.transpose(3, 0, 1, 2, 4) * WSC).astype(e4)

    host = {}
    host["wqkv1"] = wqkv_pack(g["Wq1"], g["Wk1"], g["Wv1"])
    host["wqkv2"] = wqkv_pack(g["Wq2"], g["Wk2"], g["Wv2"])
    host["wo1t"] = np.ascontiguousarray(
        g["Wo1"].T.reshape(8, 128, D).transpose(1, 0, 2) * WSC).astype(e4)
    host["wo2t"] = np.ascontiguousarray(
        g["Wo2"].T.reshape(8, 128, D).transpose(1, 0, 2) * WSC).astype(e4)
    host["e1w"] = np.ascontiguousarray(
        g["E1"].T.reshape(8, 128, 4, 128).transpose(1, 0, 2, 3) * WSC).astype(e4)
    host["d1w"] = np.ascontiguousarray(
        g["D1"].T.reshape(4, 128, 8, 128).transpose(1, 0, 2, 3) * WSC).astype(e4)
    host["e2w"] = np.ascontiguousarray(
        g["E2"].T.reshape(8, 128, 4, 128).transpose(1, 0, 2, 3) * WSC).astype(e4)
    host["d2w"] = np.ascontiguousarray(
        g["D2"].T.reshape(4, 128, D).transpose(1, 0, 2) * WSC).astype(e4)
    mask = np.where(np.arange(DQ)[None, :] <= np.arange(128)[:, None],
                    0.0, NEG).astype(np.float32)
    host["mask4"] = np.tile(mask, (1, 4))
    if affine:
        host["grep"] = np.stack([
            np.broadcast_to(g[n].astype(np.float32), (128, D))
            for n in ("g1", "b1", "g2", "b2", "g3", "b3")]).copy()

    in_maps = []
    y = g["y"].astype(np.float32)
    mem = g["mem"].astype(np.float32)
    for c in range(N_CORES):
        sl = slice(BPC * c, BPC * (c + 1))
        m = dict(host)
        m["y0b"] = y[sl].astype(bf)
        yT = np.ascontiguousarray(y[sl].transpose(0, 2, 1)).astype(e4)
        m["y0T"] = np.ascontiguousarray(
            yT.reshape(BPC, 8, 128, S_T).transpose(0, 2, 1, 3))
        mT = mem[sl].transpose(0, 2, 1).astype(e4)  # [b, D, S_M]
        # [b, k, p, j, i, q] -> [b, j, p, i, k, q]
        m["memTp"] = np.ascontiguousarray(
            mT.reshape(BPC, 8, 128, 8, 2, 128).transpose(0, 3, 2, 4, 1, 5))
        in_maps.append(m)
    return in_maps, affine


def kernel(**inputs):
    in_maps, affine = _prep_host(inputs)
    if affine not in _CACHE:
        _CACHE[affine] = _build(affine)
    nc = _CACHE[affine]
    res = run_bass_kernel_spmd(nc, in_maps, list(range(N_CORES)))
    return np.concatenate([r["out"] for r in res.results], axis=0)


if __name__ == "__main__":
    rng = np.random.default_rng(0)
    ins = {
        "mem": rng.standard_normal((B, S_M, D), dtype=np.float32),
        "y": rng.standard_normal((B, S_T, D), dtype=np.float32),
        **{k: (rng.standard_normal(s, dtype=np.float32) * 0.02).astype(np.float32)
           for k, s in {
               "Wq1": (H, D, DQ), "Wk1": (H, D, DQ), "Wv1": (H, D, DQ),
               "Wo1": (D, D), "Wq2": (H, D, DQ), "Wk2": (H, D, DQ),
               "Wv2": (H, D, DQ), "Wo2": (D, D), "E1": (BNK, D),
               "D1": (HID, BNK), "E2": (BNK, HID), "D2": (D, BNK)}.items()},
        "g1": np.ones(D, np.float32), "b1": np.zeros(D, np.float32),
        "g2": np.ones(D, np.float32), "b2": np.zeros(D, np.float32),
        "g3": np.ones(D, np.float32), "b3": np.zeros(D, np.float32),
    }
    o = kernel(**ins)
    print("out", o.shape, o.dtype, np.abs(o).mean())


# revision 35
# speedup vs baseline: 1.0466x; 1.0125x over previous
# Trainium2 Bass kernel for nn_DecoderBlock (masked self-attn + cross-attn +
# LFFN decoder block with "linear" softmax attention over the head dim).
#
# Sharding: data-parallel over batch — 16 batch elems / 8 cores = 2 per core.
# All weights replicated per core (bf16); activations stream per batch elem.
#
# Math per core/batch elem (validated against the jax reference in numpy):
#   per head: Q/K/V = x @ W[h]        ([s, dq] layout, s on partitions)
#   expQ/expK = exp((Q|K)/DQ**0.25)   (mask added to Q rows < 127 first)
#   V' = V * (1/rowsum(expK))         (folds K-softmax denominator)
#   A  = expK^T @ V'                  ([dq, dq])
#   softQ = expQ * (1/rowsum(expQ));  softQT = PE-transpose(softQ)   [dq, s]
#   BmT = A^T @ softQT                ([dq, s])
#   out rows [128h:128h+128] = sum_j BmT[:, j::8].T @ Wo.T[128j:128j+128, :]
#     (replicates the module's raw [b,h,s,d] -> [b, s, h*d] view)
#   residual + layernorm in natural [s, D] layout; transposed copy of the LN
#   output is produced on the PE for the next phase's lhsT operands.
#
# All weights are host-packed into [128, ...] images so each group loads with
# ONE big DMA; all transposes run on the TensorE (identity matmul) instead of
# the descriptor-bound DMA-transpose path.
import numpy as np
import ml_dtypes

import concourse.bacc as bacc
import concourse.mybir as mybir
import concourse.tile as tile
from concourse.bass_utils import run_bass_kernel_spmd
from concourse.masks import make_identity

H, D, DQ, BNK, HID = 8, 1024, 128, 512, 1024
B, S_T, S_M = 16, 1024, 2048
SCALE = DQ ** 0.25
EPS = 1e-5
NEG = -200.0 * 16  # pre-scaled: Q psum carries 16*Q
N_CORES = 8
BPC = B // N_CORES  # batch elems per core

f32 = mybir.dt.float32
bf16 = mybir.dt.bfloat16
fp8 = mybir.dt.float8e4
PM_DR = mybir.MatmulPerfMode.DoubleRow
AF = mybir.ActivationFunctionType
ALU = mybir.AluOpType
bf = ml_dtypes.bfloat16
e4 = ml_dtypes.float8_e4m3fn
WSC = 16.0  # host weight scale into fp8 range


def _build(affine: bool):
    nc = bacc.Bacc("TRN2", target_bir_lowering=False, debug=False,
                   enable_asserts=True, num_devices=N_CORES)

    def din(name, shape, dt=fp8):
        return nc.dram_tensor(name, list(shape), dt, kind="ExternalInput").ap()

    y0b = din("y0b", [BPC, S_T, D], bf16)            # natural bf16 (residual)
    y0T = din("y0T", [BPC, 128, 8, S_T])             # [b][128][kchunk][S_T]
    memTp = din("memTp", [BPC, 8, 128, 2, 8, 128])   # [b][jpair][p][i][k][q]
    wqkv1 = din("wqkv1", [128, 3, 2, 8, 512])        # [p][qkv][hg][kchunk][512]
    wqkv2 = din("wqkv2", [128, 3, 2, 8, 512])
    wo1t = din("wo1t", [128, 8, D])                  # [p][j][D]
    wo2t = din("wo2t", [128, 8, D])
    e1w = din("e1w", [128, 8, 4, 128])               # [p][kchunk][bn_tile][q]
    d1w = din("d1w", [128, 4, 8, 128])               # [p][bn_chunk][hid_tile][q]
    e2w = din("e2w", [128, 8, 4, 128])               # [p][hid_chunk][bn_tile][q]
    d2w = din("d2w", [128, 4, D])                    # [p][bn_chunk][D]
    mask4 = din("mask4", [128, 512], f32)
    grep = din("grep", [6, 128, D], f32) if affine else None

    out = nc.dram_tensor("out", [BPC, S_T, D], f32, kind="ExternalOutput").ap()

    with tile.TileContext(nc) as tc:
        with tc.tile_pool(name="dram", bufs=1, space="DRAM") as dpool:
            y1d = dpool.tile([BPC, S_T, D], bf16)
            y2d = dpool.tile([BPC, S_T, D], bf16)

            with tc.tile_pool(name="consts", bufs=1) as cpool:
                maskt = cpool.tile([128, 512], f32, tag="maskt")
                nc.sync.dma_start(maskt[:], mask4[:])
                eps_t = cpool.tile([128, 1], f32, tag="eps_t")
                nc.vector.memset(eps_t[:], EPS)
                ident = cpool.tile([128, 128], bf16, tag="ident")
                make_identity(nc, ident[:])
                gb = None
                if affine:
                    gb = [cpool.tile([128, D], f32, tag=f"gb{i}", name=f"gb{i}")
                          for i in range(6)]
                    for i in range(6):
                        nc.sync.dma_start(gb[i][:], grep[i])

                # persistent transposed-activation pool: one [128, 8, S_T]
                # tile per generation, 3 rotating buffers (y1T b0, y1T b1,
                # y2T b0; y2T b1 reuses y1T b0's buffer after last read)
                with tc.tile_pool(name="xT", bufs=1) as xpool, \
                     tc.tile_pool(name="wstg", bufs=1) as wpool:
                    def xt_alloc():
                        return xpool.tile([128, 8, S_T], fp8, tag="xT",
                                          name="xT", bufs=3)

                    def stg_kv(wqkv):
                        # staged K/V slab for the NEXT attn phase; the load
                        # overlaps the previous phase (tile WAR, not pool
                        # barrier, orders it)
                        stg = wpool.tile([128, 16384], fp8, tag="wstage",
                                         name="wstage", bufs=1)
                        kv = stg[:].rearrange("p (x h k q) -> p x h k q",
                                              x=2, h=2, k=8)
                        nc.sync.dma_start(kv[:, :, 0], wqkv[:, 1:3, 0])
                        nc.sync.dma_start(kv[:, :, 1], wqkv[:, 1:3, 1])
                        return kv

                    y1T = [None] * BPC
                    y2T = [None] * BPC
                    ctx = dict(nc=nc, tc=tc, maskt=maskt, eps_t=eps_t,
                               ident=ident, gb=gb)

                    kv1 = stg_kv(wqkv1)
                    _phase_attn(ctx, masked=True, xq_dram=y0T, memT=None,
                                wqkv=wqkv1, wot=wo1t, kv=kv1, res_d=y0b,
                                y_next_d=y1d, xT_in=None, xT_out=y1T,
                                xt_alloc=xt_alloc, gbi=0)
                    kv2 = stg_kv(wqkv2)
                    _phase_attn(ctx, masked=False, xq_dram=None, memT=memTp,
                                wqkv=wqkv2, wot=wo2t, kv=kv2, res_d=y1d,
                                y_next_d=y2d, xT_in=y1T, xT_out=y2T,
                                xt_alloc=xt_alloc, gbi=2)
                    stg = wpool.tile([128, 16384], fp8, tag="wstage",
                                     name="wstage", bufs=1)
                    e1v = stg[:, 0:4096].rearrange("p (k t q) -> p k t q",
                                                   k=8, t=4)
                    d1v = stg[:, 4096:8192].rearrange("p (c t q) -> p c t q",
                                                      c=4, t=8)
                    nc.sync.dma_start(e1v, e1w[:])
                    nc.sync.dma_start(d1v, d1w[:])
                    _phase_lffn(ctx, y2T, e1v, d1v, e2w, d2w, y2d, out, gbi=4)

    nc.compile()
    return nc


def _layernorm(ctx, pool, rsd, dst_dram, gbi, out_dt):
    """LN over the free axis of rsd [128, D] f32; write `out_dt` tile to
    dst_dram and return the SBUF tile."""
    nc, eps_t, gb = ctx["nc"], ctx["eps_t"], ctx["gb"]
    st6 = pool.tile([128, 2, 6], f32, tag="ln_st6", bufs=2)
    mv = pool.tile([128, 2], f32, tag="ln_mv", bufs=2)
    nc.vector.bn_stats(st6[:, 0, :], rsd[:, 0:512])
    nc.vector.bn_stats(st6[:, 1, :], rsd[:, 512:1024])
    nc.vector.bn_aggr(mv[:], st6[:])
    sd = pool.tile([128, 1], f32, tag="ln_sd", bufs=2)
    nc.scalar.activation(sd[:], mv[:, 1:2], AF.Sqrt, bias=eps_t[:])
    rstd = pool.tile([128, 1], f32, tag="ln_rstd", bufs=2)
    nc.vector.reciprocal(rstd[:], sd[:])
    cneg = pool.tile([128, 1], f32, tag="ln_cneg", bufs=2)
    nc.vector.scalar_tensor_tensor(
        out=cneg[:], in0=mv[:, 0:1], scalar=-1.0, in1=rstd[:],
        op0=ALU.mult, op1=ALU.mult)
    yt = pool.tile([128, D], out_dt, tag="ln_out", bufs=4)
    nc.scalar.activation(yt[:], rsd[:], AF.Identity, scale=rstd[:], bias=cneg[:])
    if gb is not None:
        g_t, b_t = gb[gbi], gb[gbi + 1]
        nc.vector.tensor_tensor(out=yt[:], in0=yt[:], in1=g_t[:], op=ALU.mult)
        nc.vector.tensor_tensor(out=yt[:], in0=yt[:], in1=b_t[:], op=ALU.add)
    nc.scalar.dma_start(dst_dram, yt[:])
    return yt


def _phase_attn(ctx, masked, xq_dram, memT, wqkv, wot, kv, res_d, y_next_d,
                xT_in, xT_out, xt_alloc, gbi):
    """One attention phase (self or cross) for all batch elems.

    Weights load on the sync HWDGE ring (K/V slab first so stage A can start
    early); activations/residuals use the scalar ring so the two FIFOs don't
    serialize each other.
    """
    nc, tc, ident = ctx["nc"], ctx["tc"], ctx["ident"]
    n_kv = 8 if memT is None else 16
    with tc.tile_pool(name="attn_sb", bufs=1) as sb:
        wq = sb.tile([128, 2, 8, 512], fp8, tag="wq")
        nc.sync.dma_start(wq[:], wqkv[:, 0])
        wo = sb.tile([128, 8, D], fp8, tag="wo")
        nc.sync.dma_start(wo[:], wot[:])

        xqs = [None] * BPC
        if xq_dram is not None:
            for b in range(BPC):
                xqs[b] = sb.tile([128, 8, S_T], fp8, tag="xq", bufs=2,
                                 name="xq")
                nc.scalar.dma_start(xqs[b][:], xq_dram[b])
        else:
            xqs = xT_in

        with tc.tile_pool(name="attn_ps", bufs=1, space="PSUM") as ps:
            for b in range(BPC):
                xq = xqs[b]

                xt_next = xt_alloc()
                xT_out[b] = xt_next
                for hg in range(2):
                    # ---- stage A: K/V proj + exp/evac + A accumulation ----
                    expk = sb.tile([128, n_kv, 512], bf16, tag="expk")
                    expv = sb.tile([128, n_kv, 512], bf16, tag="expv")
                    for j in range(n_kv // 2):
                        if memT is not None:
                            mt = sb.tile([128, 2, 8, 128], fp8, tag="mt",
                                         bufs=4)
                            nc.gpsimd.dma_start(mt[:], memT[b, j])
                        for i in range(2):
                            sm = 2 * j + i
                            kps = ps.tile([128, 512], f32, tag="ps512", bufs=3)
                            vps = ps.tile([128, 512], f32, tag="ps512", bufs=3)
                            for k in range(0, 8, 2):
                                if memT is None:
                                    lhsT = xq[:, k:k + 2, 128 * sm:128 * (sm + 1)]
                                else:
                                    lhsT = mt[:, i, k:k + 2, :]
                                nc.tensor.matmul(kps[:], lhsT,
                                                 kv[:, 0, hg, k:k + 2, :],
                                                 start=(k == 0), stop=(k == 6),
                                                 perf_mode=PM_DR)
                                nc.tensor.matmul(vps[:], lhsT,
                                                 kv[:, 1, hg, k:k + 2, :],
                                                 start=(k == 0), stop=(k == 6),
                                                 perf_mode=PM_DR)
                            nc.scalar.activation(expk[:, sm, :], kps[:], AF.Exp,
                                                 scale=1.0 / (WSC * SCALE))
                            krs = sb.tile([128, 4], f32, tag="krs", bufs=2)
                            nc.vector.tensor_reduce(
                                out=krs[:],
                                in_=expk[:, sm, :].rearrange("p (h q) -> p h q", h=4),
                                axis=mybir.AxisListType.X, op=ALU.add)
                            krr = sb.tile([128, 4], f32, tag="krr", bufs=2)
                            nc.vector.reciprocal(krr[:], krs[:])
                            nc.vector.tensor_tensor(
                                out=expv[:, sm, :].rearrange("p (h q) -> p h q", h=4),
                                in0=vps[:].rearrange("p (h q) -> p h q", h=4),
                                in1=krr[:].unsqueeze(2).broadcast_to([128, 4, 128]),
                                op=ALU.mult)
                    # ---- stage B Q proj (fills PE while stage-A evacs
                    # drain), then the A accumulation, then transposes ----
                    softqT = sb.tile([128, 4, S_T], bf16, tag="softqT", bufs=1)
                    sqa = sb.tile([128, 8, 4, 128], bf16, tag="sqa", bufs=1)
                    for st in range(8):
                        qps = ps.tile([128, 512], f32, tag="ps512", bufs=3)
                        for k in range(0, 8, 2):
                            nc.tensor.matmul(
                                qps[:], xq[:, k:k + 2, 128 * st:128 * (st + 1)],
                                wq[:, hg, k:k + 2, :], start=(k == 0),
                                stop=(k == 6), perf_mode=PM_DR)
                        if masked and st == 0:
                            nc.vector.tensor_tensor(
                                out=qps[:], in0=qps[:], in1=ctx["maskt"][:],
                                op=ALU.add)
                        eq = sb.tile([128, 512], f32, tag="eq", bufs=3)
                        nc.scalar.activation(eq[:], qps[:], AF.Exp,
                                             scale=1.0 / (WSC * SCALE))
                        qrs = sb.tile([128, 4], f32, tag="qrs", bufs=2)
                        nc.vector.tensor_reduce(
                            out=qrs[:], in_=eq[:].rearrange("p (h q) -> p h q", h=4),
                            axis=mybir.AxisListType.X, op=ALU.add)
                        qrr = sb.tile([128, 4], f32, tag="qrr", bufs=2)
                        nc.vector.reciprocal(qrr[:], qrs[:])
                        nc.vector.tensor_tensor(
                            out=sqa[:, st], in0=eq[:].rearrange("p (h q) -> p h q", h=4),
                            in1=qrr[:].unsqueeze(2).broadcast_to([128, 4, 128]),
                            op=ALU.mult)
                    # per-region accumulation groups must stay consecutive:
                    # interleaving groups within one PSUM bank corrupts them.
                    aps = ps.tile([128, 512], f32, tag="aps", bufs=1)
                    for hi in range(4):
                        for sm in range(n_kv):
                            nc.tensor.matmul(
                                aps[:, 128 * hi:128 * (hi + 1)],
                                expk[:, sm, 128 * hi:128 * (hi + 1)],
                                expv[:, sm, 128 * hi:128 * (hi + 1)],
                                start=(sm == 0), stop=(sm == n_kv - 1))
                    asb = sb.tile([128, 512], bf16, tag="asb", bufs=2)
                    nc.scalar.activation(asb[:], aps[:], AF.Identity)
                    for st in range(8):
                        tp = ps.tile([128, 8, 128], bf16, tag="tpb", bufs=2)
                        for hi in range(4):
                            nc.tensor.transpose(tp[:, hi, :], sqa[:, st, hi, :],
                                                ident[:])
                        nc.scalar.activation(
                            softqT[:, :, 128 * st:128 * (st + 1)],
                            tp[:, 0:4, :], AF.Identity)

                    # ---- stage C: Bm, Wo, residual + LN per head ----
                    nats, ybs = [], []
                    def _load_nat(hi):
                        hb = 4 * hg + hi
                        nat = sb.tile([128, D], bf16, tag="res_nat", bufs=3,
                                      name="res_nat")
                        nc.scalar.dma_start(
                            nat[:], res_d[b, 128 * hb:128 * (hb + 1), :])
                        nats.append(nat)
                    for hi in range(3):
                        _load_nat(hi)
                    for hi in range(4):
                        hb = 4 * hg + hi  # head == output s-tile block
                        # bms2[p, jj, m] = BmT(16x) at s = 8*m + jj, so the
                        # Wo contraction can pair jj-groups for DoubleRow
                        bms = sb.tile([128, 8, 128], fp8, tag="bms", bufs=2)
                        for half in range(2):
                            bmt = ps.tile([128, 512], f32, tag="ps512", bufs=3)
                            nc.tensor.matmul(bmt[:],
                                             asb[:, 128 * hi:128 * (hi + 1)],
                                             softqT[:, hi,
                                                    512 * half:512 * (half + 1)])
                            nc.vector.tensor_copy(
                                bms[:, :, 64 * half:64 * (half + 1)],
                                bmt[:].rearrange("p (m j) -> p j m", j=8))
                        ops = ps.tile([128, D], f32, tag="ps1k", bufs=1)
                        for jj in range(0, 8, 2):
                            for nh in range(2):
                                nc.tensor.matmul(
                                    ops[:, 512 * nh:512 * (nh + 1)],
                                    bms[:, jj:jj + 2, :],
                                    wo[:, jj:jj + 2, 512 * nh:512 * (nh + 1)],
                                    start=(jj == 0), stop=(jj == 6),
                                    perf_mode=PM_DR)
                        rsd = sb.tile([128, D], f32, tag="rsd", bufs=1)
                        nc.vector.scalar_tensor_tensor(
                            out=rsd[:], in0=ops[:], scalar=1.0 / (WSC * WSC),
                            in1=nats[hi][:], op0=ALU.mult, op1=ALU.add)
                        if hi == 0:
                            _load_nat(3)
                        yb = _layernorm(ctx, sb, rsd,
                                        y_next_d[b, 128 * hb:128 * (hb + 1), :],
                                        gbi, bf16)
                        ybs.append(yb)
                    # transposed copies for the next phase, batched so the
                    # PE never waits on an LN chain mid-stage
                    for hi in range(4):
                        hb = 4 * hg + hi
                        tp2 = ps.tile([128, 8, 128], bf16, tag="tpb", bufs=2)
                        for k in range(8):
                            nc.tensor.transpose(
                                tp2[:, k, :],
                                ybs[hi][:, 128 * k:128 * (k + 1)], ident[:])
                        nc.scalar.activation(
                            xt_next[:, :, 128 * hb:128 * (hb + 1)], tp2[:],
                            AF.Identity)


def _phase_lffn(ctx, y2T, e1w_d, d1w_d, e2w_d2, d2w_d2, y2d, out, gbi):
    nc, tc = ctx["nc"], ctx["tc"]
    with tc.tile_pool(name="ffn_sb", bufs=1) as sb:
        e1, d1 = e1w_d, d1w_d  # staged SBUF views
        e2 = sb.tile([128, 8, 4, 128], fp8, tag="e2")
        nc.sync.dma_start(e2[:], e2w_d2[:])
        d2 = sb.tile([128, 4, D], fp8, tag="d2")
        nc.sync.dma_start(d2[:], d2w_d2[:])

        with tc.tile_pool(name="ffn_ps", bufs=1, space="PSUM") as ps:
            for b in range(BPC):
                xT = y2T[b]  # [128, 8, S_T] fp8
                # h1T = E1 @ y2T  (16x scale)  [BN(4), S_T]
                h1T = sb.tile([128, 4, S_T], fp8, tag="h1T", bufs=2)
                for t_ in range(4):
                    acc = ps.tile([128, S_T], f32, tag="acc", bufs=2)
                    for nh in range(2):
                        for k in range(0, 8, 2):
                            nc.tensor.matmul(
                                acc[:, 512 * nh:512 * (nh + 1)],
                                e1[:, k:k + 2, t_, :],
                                xT[:, k:k + 2, 512 * nh:512 * (nh + 1)],
                                start=(k == 0), stop=(k == 6), perf_mode=PM_DR)
                    nc.scalar.activation(h1T[:, t_], acc[:], AF.Identity)
                # h2T = D1 @ h1T (256x) -> silu(x/256) -> swT (1x)
                swT = sb.tile([128, 8, S_T], fp8, tag="swT", bufs=2)
                for t_ in range(8):
                    acc = ps.tile([128, S_T], f32, tag="acc", bufs=2)
                    for nh in range(2):
                        for k in range(0, 4, 2):
                            nc.tensor.matmul(
                                acc[:, 512 * nh:512 * (nh + 1)],
                                d1[:, k:k + 2, t_, :],
                                h1T[:, k:k + 2, 512 * nh:512 * (nh + 1)],
                                start=(k == 0), stop=(k == 2), perf_mode=PM_DR)
                    nc.scalar.activation(swT[:, t_], acc[:], AF.Silu,
                                         scale=1.0 / (WSC ** 2))
                # g1T = E2 @ swT  (16x)
                g1T = sb.tile([128, 4, S_T], fp8, tag="g1T", bufs=2)
                for t_ in range(4):
                    acc = ps.tile([128, S_T], f32, tag="acc", bufs=2)
                    for nh in range(2):
                        for k in range(0, 8, 2):
                            nc.tensor.matmul(
                                acc[:, 512 * nh:512 * (nh + 1)],
                                e2[:, k:k + 2, t_, :],
                                swT[:, k:k + 2, 512 * nh:512 * (nh + 1)],
                                start=(k == 0), stop=(k == 6), perf_mode=PM_DR)
                    nc.scalar.activation(g1T[:, t_], acc[:], AF.Identity)
                # ffn[st] = g1T[:, st].T @ D2T (256x); residual; LN3 -> out
                for st in range(8):
                    nat = sb.tile([128, D], bf16, tag="y2res", bufs=2)
                    nc.scalar.dma_start(nat[:],
                                        y2d[b, 128 * st:128 * (st + 1), :])
                    acc = ps.tile([128, D], f32, tag="acc2", bufs=2)
                    for nh in range(2):
                        for k in range(0, 4, 2):
                            nc.tensor.matmul(
                                acc[:, 512 * nh:512 * (nh + 1)],
                                g1T[:, k:k + 2, 128 * st:128 * (st + 1)],
                                d2[:, k:k + 2, 512 * nh:512 * (nh + 1)],
                                start=(k == 0), stop=(k == 2), perf_mode=PM_DR)
                    rsd = sb.tile([128, D], f32, tag="rsd", bufs=2)
                    nc.vector.scalar_tensor_tensor(
                        out=rsd[:], in0=acc[:], scalar=1.0 / (WSC ** 2),
                        in1=nat[:], op0=ALU.mult, op1=ALU.add)
                    _layernorm(ctx, sb, rsd,
                               out[b, 128 * st:128 * (st + 1), :], gbi, f32)


_CACHE = {}


def _prep_host(inputs):
    """Convert/transpose/pack weights + activations per the kernel layout."""
    g = {k: np.asarray(v) for k, v in inputs.items()}
    affine = not (
        np.all(g["g1"] == 1) and np.all(g["g2"] == 1) and np.all(g["g3"] == 1)
        and np.all(g["b1"] == 0) and np.all(g["b2"] == 0) and np.all(g["b3"] == 0))

    def wqkv_pack(q, k, v):
        # [H, D, DQ] -> [p=128][qkv][hg][kchunk][512] (4 heads concat)
        def onev2(w):
            arr = np.empty((2, 8, 128, 512), np.float32)
            for hg in range(2):
                for kc in range(8):
                    cols = [w[4 * hg + hi, 128 * kc:128 * (kc + 1), :]
                            for hi in range(4)]
                    arr[hg, kc] = np.concatenate(cols, axis=1)
            return arr
        st = np.stack([onev2(q), onev2(k), onev2(v)])  # [3,2,8,128,512]
        return np.ascontiguousarray(st.transpose(3, 0, 1, 2, 4) * WSC).astype(e4)

    host = {}
    host["wqkv1"] = wqkv_pack(g["Wq1"], g["Wk1"], g["Wv1"])
    host["wqkv2"] = wqkv_pack(g["Wq2"], g["Wk2"], g["Wv2"])
    host["wo1t"] = np.ascontiguousarray(
        g["Wo1"].T.reshape(8, 128, D).transpose(1, 0, 2) * WSC).astype(e4)
    host["wo2t"] = np.ascontiguousarray(
        g["Wo2"].T.reshape(8, 128, D).transpose(1, 0, 2) * WSC).astype(e4)
    host["e1w"] = np.ascontiguousarray(
        g["E1"].T.reshape(8, 128, 4, 128).transpose(1, 0, 2, 3) * WSC).astype(e4)
    host["d1w"] = np.ascontiguousarray(
        g["D1"].T.reshape(4, 128, 8, 128).transpose(1, 0, 2, 3) * WSC).astype(e4)
    host["e2w"] = np.ascontiguousarray(
        g["E2"].T.reshape(8, 128, 4, 128).transpose(1, 0, 2, 3) * WSC).astype(e4)
    host["d2w"] = np.ascontiguousarray(
        g["D2"].T.reshape(4, 128, D).transpose(1, 0, 2) * WSC).astype(e4)
    mask = np.where(np.arange(DQ)[None, :] <= np.arange(128)[:, None],
                    0.0, NEG).astype(np.float32)
    host["mask4"] = np.tile(mask, (1, 4))
    if affine:
        host["grep"] = np.stack([
            np.broadcast_to(g[n].astype(np.float32), (128, D))
            for n in ("g1", "b1", "g2", "b2", "g3", "b3")]).copy()

    in_maps = []
    y = g["y"].astype(np.float32)
    mem = g["mem"].astype(np.float32)
    for c in range(N_CORES):
        sl = slice(BPC * c, BPC * (c + 1))
        m = dict(host)
        m["y0b"] = y[sl].astype(bf)
        yT = np.ascontiguousarray(y[sl].transpose(0, 2, 1)).astype(e4)
        m["y0T"] = np.ascontiguousarray(
            yT.reshape(BPC, 8, 128, S_T).transpose(0, 2, 1, 3))
        mT = mem[sl].transpose(0, 2, 1).astype(e4)  # [b, D, S_M]
        # [b, k, p, j, i, q] -> [b, j, p, i, k, q]
        m["memTp"] = np.ascontiguousarray(
            mT.reshape(BPC, 8, 128, 8, 2, 128).transpose(0, 3, 2, 4, 1, 5))
        in_maps.append(m)
    return in_maps, affine


def kernel(**inputs):
    in_maps, affine = _prep_host(inputs)
    if affine not in _CACHE:
        _CACHE[affine] = _build(affine)
    nc = _CACHE[affine]
    res = run_bass_kernel_spmd(nc, in_maps, list(range(N_CORES)))
    return np.concatenate([r["out"] for r in res.results], axis=0)


if __name__ == "__main__":
    rng = np.random.default_rng(0)
    ins = {
        "mem": rng.standard_normal((B, S_M, D), dtype=np.float32),
        "y": rng.standard_normal((B, S_T, D), dtype=np.float32),
        **{k: (rng.standard_normal(s, dtype=np.float32) * 0.02).astype(np.float32)
           for k, s in {
               "Wq1": (H, D, DQ), "Wk1": (H, D, DQ), "Wv1": (H, D, DQ),
               "Wo1": (D, D), "Wq2": (H, D, DQ), "Wk2": (H, D, DQ),
               "Wv2": (H, D, DQ), "Wo2": (D, D), "E1": (BNK, D),
               "D1": (HID, BNK), "E2": (BNK, HID), "D2": (D, BNK)}.items()},
        "g1": np.ones(D, np.float32), "b1": np.zeros(D, np.float32),
        "g2": np.ones(D, np.float32), "b2": np.zeros(D, np.float32),
        "g3": np.ones(D, np.float32), "b3": np.zeros(D, np.float32),
    }
    o = kernel(**ins)
    print("out", o.shape, o.dtype, np.abs(o).mean())
